# revision 1
# baseline (speedup 1.0000x reference)
"""Neural Tensor Network (NTN) scoring kernel for Trainium2 (Bass/Tile).

score_k(e1, e2, r) = u_k . tanh( e1^T W[r,k] e2 + v_k . [e1;e2] + b_k )
pred = sigmoid( sum_k score_k )

Strategy
--------
Host: sort the batch by relation id, split the sorted order into 8 chunks of
512 (data-parallel over batch; each core's chunk covers a contiguous relation
range, i.e. the relation tables are sharded by relation id). All per-relation
parameters are folded into one augmented table XT[r] of shape [101, 408] such
that with e1~ = [e1; 1]:

    P = e1~^T @ XT[r]                      # a single matmul per relation
    P[k*101 + j]   = (e1^T W_k)[j] + v_k^b[j]     (j < 100)
    P[k*101 + 100] = v_k^a . e1 + b_k
    P[404 + k]     = u_k

so    g_pre_k = sum_{j<=100} P[k*101+j] * e2~[j]    (e2~ = [e2; 1])
      pred    = sigmoid( sum_k P[404+k] * tanh(g_pre_k) )

Items sharing a relation form groups; groups are packed into 32-item slots
(4 slots per 128-row block; PE column-group granularity is 32). The host
emits the core's slot-ordered XT shard so the device streams one [101, 408]
tile per slot (the full f32 table traffic stays on the device; routing /
sharding is host work).

Device (one SPMD program on 8 cores):
  * heads/tails entity rows are looked up on-device: two dense indirect-DMA
    gathers (one row per batch item) + one indirect scatter through a DRAM
    bounce buffer put the rows into padded slot order,
  * per block: PE transposes the e1 rows, four matmuls (one per slot, each
    [101,32]^T @ [101,408], packed into the four 32-partition column strips
    of one PSUM tile) produce P for all 128 rows,
  * VectorE computes the segmented e2~ reduction, ScalarE applies tanh, a
    fused multiply-reduce applies the u-weights, ScalarE applies sigmoid.
"""

import sys
from contextlib import ExitStack

for _p in ("/opt/trn_rl_repo", "/opt/trn_rl_repo/concourse"):
    if _p not in sys.path:
        sys.path.insert(0, _p)

import numpy as np  # noqa: E402

import concourse.bass as bass  # noqa: E402
import concourse.mybir as mybir  # noqa: E402
import concourse.tile as tile  # noqa: E402
from concourse.bass import IndirectOffsetOnAxis  # noqa: E402
from concourse.masks import make_identity  # noqa: E402

F32 = mybir.dt.float32
I32 = mybir.dt.int32

B = 4096
D = 100
K = 4
NREL = 1000
NENT = 100000
NCORES = 8
CHUNK = B // NCORES
DA = D + 1          # augmented contraction dim (e1; 1)
NW = K * DA         # 404 folded W/V/B columns
NX = NW + K         # 408 = + u columns
SLOT = 32           # items per slot (PE col-group granularity)
DCOL = CHUNK // 128  # 4 dense columns per core


# ---------------------------------------------------------------------------
# Walrus on this toolchain rejects instructions carrying more than one
# sync-wait command. After Tile schedules, move any excess waits onto
# freshly inserted same-engine nops placed directly before the instruction
# (engines execute their stream in order, so semantics are unchanged).
# ---------------------------------------------------------------------------
_WAIT_LIMIT = 1
_split_counter = [0]


def _split_excess_waits(nc):
    for f in nc.m.functions:
        for blk in f.blocks:
            il = blk.instructions
            k = 0
            while k < len(il):
                inst = il[k]
                si = inst.sync_info
                if si is not None and si.on_wait and len(si.on_wait) > _WAIT_LIMIT:
                    waits = list(si.on_wait)
                    excess = waits[:-_WAIT_LIMIT]
                    del si.on_wait[:-_WAIT_LIMIT]
                    for w in excess:
                        _split_counter[0] += 1
                        nop = mybir.InstNoOp(
                            name=f"I-waitsplit-{_split_counter[0]}", ins=[], outs=[])
                        nop.engine = inst.engine
                        nop.sync_info = mybir.SyncInfo(on_wait=[w], on_update=[])
                        nc.register_instruction(nop, overwrite=True)
                        il.insert(k, nop)
                        k += 1
                k += 1


_orig_tile_exit = tile.TileContext.__exit__


def _patched_tile_exit(self, exc_type, exc, tb):
    r = _orig_tile_exit(self, exc_type, exc, tb)
    if exc_type is None:
        _split_excess_waits(self.nc)
    return r


if getattr(tile.TileContext, "_ant_wait_split_patch", False) is False:
    tile.TileContext.__exit__ = _patched_tile_exit
    tile.TileContext._ant_wait_split_patch = True


# ---------------------------------------------------------------------------
# Host-side preparation
# ---------------------------------------------------------------------------
def _build_xt(W, V, Bp, U):
    """Fold W/V/Bp/U into the augmented relation table XT [NREL, 101, 408]."""
    XT = np.zeros((NREL, DA, NX), np.float32)
    core = np.zeros((NREL, DA, K, DA), np.float32)
    core[:, :D, :, :D] = W.transpose(0, 2, 1, 3)          # [r, d, k, e]
    core[:, D, :, :D] = V[:, :, D:]                        # v^b
    core[:, :D, :, D] = V[:, :, :D].transpose(0, 2, 1)     # v^a
    core[:, D, :, D] = Bp
    XT[:, :, :NW] = core.reshape(NREL, DA, NW)
    XT[:, D, NW:NX] = U
    return XT


def _route(heads, tails, relations):
    """Sort by relation, chunk into cores, pack groups into 32-item slots."""
    order = np.argsort(relations, kind="stable")
    cores = []
    for c in range(NCORES):
        idxs = order[c * CHUNK:(c + 1) * CHUNK]
        rels = relations[idxs]
        slots = []  # (relation id, dense positions)
        i = 0
        while i < CHUNK:
            j = i
            while j < CHUNK and rels[j] == rels[i]:
                j += 1
            for a in range(i, j, SLOT):
                slots.append((int(rels[i]), np.arange(a, min(a + SLOT, j))))
            i = j
        cores.append((idxs, slots))

    S = max(len(c[1]) for c in cores)
    NBLK = (S + 3) // 4
    S = NBLK * 4

    routed = []
    for c in range(NCORES):
        idxs, slots = cores[c]
        slot_rels = np.zeros(S, np.int64)
        hsd = np.zeros((128, DCOL), np.int32)
        tsd = np.zeros((128, DCOL), np.int32)
        scat = np.zeros((128, DCOL), np.int32)
        placement = []  # (orig batch index, block, partition row)
        for di in range(CHUNK):
            hsd[di % 128, di // 128] = heads[idxs[di]]
            tsd[di % 128, di // 128] = tails[idxs[di]]
        for s, (rr, dense_pos) in enumerate(slots):
            slot_rels[s] = rr
            b, j = divmod(s, 4)
            for t, di in enumerate(dense_pos):
                prow = b * 128 + SLOT * j + t
                scat[di % 128, di // 128] = prow
                placement.append((int(idxs[di]), b, SLOT * j + t))
        routed.append(dict(slot_rels=slot_rels, hsd=hsd, tsd=tsd, scat=scat,
                           placement=placement))
    return routed, S, NBLK


# ---------------------------------------------------------------------------
# Device program
# ---------------------------------------------------------------------------
def _build_program(S, NBLK, xt_bufs=4, xt_chunk=8):
    nc = bass.Bass("TRN2", target_bir_lowering=False, debug=False)

    # slot-ordered relation table, stored d-major [d, slot, col] so chunked
    # fetches have a large contiguous run per partition (spreads across all
    # SDMA engines instead of collapsing onto one)
    xtc = nc.dram_tensor("xtc", [DA, S, NX], F32, kind="ExternalInput")
    ent = nc.dram_tensor("ent", [NENT, D], F32, kind="ExternalInput")
    hsd = nc.dram_tensor("hsd", [128, DCOL], I32, kind="ExternalInput")
    tsd = nc.dram_tensor("tsd", [128, DCOL], I32, kind="ExternalInput")
    scat = nc.dram_tensor("scat", [128, DCOL], I32, kind="ExternalInput")
    pred_t = nc.dram_tensor("pred_t", [NBLK, 128], F32, kind="ExternalOutput")
    gpre = nc.dram_tensor("gpre", [128, NBLK * K], F32, kind="ExternalOutput")

    with tile.TileContext(nc) as tc, ExitStack() as ctx:
        const_pool = ctx.enter_context(tc.tile_pool(name="const", bufs=1))
        dense_pool = ctx.enter_context(tc.tile_pool(name="dense", bufs=1))
        dram_pool = ctx.enter_context(tc.tile_pool(name="bounce", bufs=1,
                                                   space="DRAM"))
        e_pool = ctx.enter_context(tc.tile_pool(name="erows", bufs=3))
        e1t_pool = ctx.enter_context(tc.tile_pool(name="e1t", bufs=3))
        xt_pool = ctx.enter_context(tc.tile_pool(name="xtrows", bufs=xt_bufs))
        tmp_pool = ctx.enter_context(tc.tile_pool(name="tmp", bufs=2))
        small_pool = ctx.enter_context(tc.tile_pool(name="small", bufs=2))
        acc_pool = ctx.enter_context(tc.tile_pool(name="acc", bufs=1))
        psum_p = ctx.enter_context(tc.tile_pool(name="pacc", bufs=2, space="PSUM"))
        psum_t = ctx.enter_context(tc.tile_pool(name="ptrans", bufs=2, space="PSUM"))
        psum_o = ctx.enter_context(tc.tile_pool(name="pout", bufs=1, space="PSUM"))

        ident = const_pool.tile([128, 128], F32)
        make_identity(nc, ident[:])

        hsd_t = const_pool.tile([128, DCOL], I32)
        nc.sync.dma_start(hsd_t[:], hsd[:])
        tsd_t = const_pool.tile([128, DCOL], I32)
        nc.sync.dma_start(tsd_t[:], tsd[:])
        scat_t = const_pool.tile([128, DCOL], I32)
        nc.sync.dma_start(scat_t[:], scat[:])

        # Dense on-device entity lookups: one gathered row per batch item,
        # laid out [e1 (0:100) | 1 | e2 (101:201) | 1] so the ones column
        # rides through the PE transpose (augmented e1~) and the e2~ AP is
        # contiguous. The rows are then scattered into padded slot order
        # through a DRAM bounce buffer, one dense column at a time (dense
        # order is block-monotone, so early blocks unblock early).
        RW = 2 * D + 2
        bounce = dram_pool.tile([NBLK * 128, RW], F32)
        zf = const_pool.tile([128, RW], F32)
        nc.vector.memset(zf[:], 0.0)
        for z in range(NBLK):
            nc.scalar.dma_start(bounce[z * 128:(z + 1) * 128, :], zf[:])

        e12 = dense_pool.tile([128, DCOL * RW], F32)
        e12v = e12[:].rearrange("p (c d) -> p c d", c=DCOL)  # [128, DCOL, RW]
        nc.vector.memset(e12v[:, :, D:DA], 1.0)
        nc.vector.memset(e12v[:, :, DA + D:RW], 1.0)
        for c in range(DCOL):
            nc.gpsimd.indirect_dma_start(
                out=e12v[:, c, 0:D], out_offset=None, in_=ent[:, :],
                in_offset=IndirectOffsetOnAxis(ap=hsd_t[:, c:c + 1], axis=0))
            nc.gpsimd.indirect_dma_start(
                out=e12v[:, c, DA:DA + D], out_offset=None, in_=ent[:, :],
                in_offset=IndirectOffsetOnAxis(ap=tsd_t[:, c:c + 1], axis=0))
            nc.gpsimd.indirect_dma_start(
                out=bounce[:, :],
                out_offset=IndirectOffsetOnAxis(ap=scat_t[:, c:c + 1], axis=0),
                in_=e12v[:, c, :], in_offset=None)

        gpre_t = acc_pool.tile([128, NBLK * K], F32)
        pred_pt = acc_pool.tile([128, NBLK], F32)
        xt_tiles = {}

        for b in range(NBLK):
            # padded-slot entity rows for this block [e1 | 1 | e2 | 1]
            ep = e_pool.tile([128, RW], F32)
            nc.sync.dma_start(ep[:], bounce[b * 128:(b + 1) * 128, :])

            # transpose the augmented heads rows -> e1~^T [101, 128]
            tp = psum_t.tile([DA, 128], F32)
            nc.tensor.transpose(out=tp[:], in_=ep[:, 0:DA], identity=ident[:])
            e1t = e1t_pool.tile([DA, 128], F32)
            nc.scalar.copy(e1t[:], tp[:])

            # four slot matmuls into the four column strips of one PSUM tile
            pacc = psum_p.tile([128, 512], F32)
            for j in range(4):
                s = 4 * b + j
                g, sl = divmod(s, xt_chunk)
                if sl == 0:  # fetch the next chunk of slot tiles
                    gn = min(xt_chunk, S - g * xt_chunk)
                    xtt = xt_pool.tile([DA, xt_chunk * NX], F32)
                    eng = nc.sync if (g % 2 == 0) else nc.scalar
                    eng.dma_start(
                        xtt[:, 0:gn * NX],
                        xtc[:, g * xt_chunk:g * xt_chunk + gn, :])
                    xt_tiles[g] = xtt
                xtt = xt_tiles[g]
                nc.tensor.matmul(
                    out=pacc[SLOT * j:SLOT * (j + 1), 0:NX],
                    lhsT=e1t[:, SLOT * j:SLOT * (j + 1)],
                    rhs=xtt[:, sl * NX:(sl + 1) * NX],
                    start=True, stop=True,
                    tile_position=(0, SLOT * j),
                )

            # g_pre = segmented sum of P * e2~  (e2 = cols 100:200, ones col 200)
            tmp = tmp_pool.tile([128, NW], F32)
            nc.vector.tensor_tensor(
                out=tmp[:].rearrange("p (k j) -> p k j", k=K),
                in0=pacc[:, 0:NW].rearrange("p (k j) -> p k j", k=K),
                in1=ep[:, DA:RW].unsqueeze(1).broadcast_to([128, K, DA]),
                op=mybir.AluOpType.mult,
            )
            nc.vector.reduce_sum(
                out=gpre_t[:, K * b:K * (b + 1)],
                in_=tmp[:].rearrange("p (k j) -> p k j", k=K),
                axis=mybir.AxisListType.X,
            )
            th = small_pool.tile([128, K], F32, tag="th")
            nc.scalar.activation(th[:], gpre_t[:, K * b:K * (b + 1)],
                                 mybir.ActivationFunctionType.Tanh)
            scr = small_pool.tile([128, K], F32, tag="scr")
            sco = small_pool.tile([128, 1], F32, tag="sco")
            nc.vector.tensor_tensor(
                out=scr[:], in0=th[:], in1=pacc[:, NW:NX],
                op=mybir.AluOpType.mult,
            )
            nc.vector.reduce_sum(out=sco[:], in_=scr[:],
                                 axis=mybir.AxisListType.X)
            nc.scalar.activation(pred_pt[:, b:b + 1], sco[:],
                                 mybir.ActivationFunctionType.Sigmoid)

        po = psum_o.tile([NBLK, 128], F32)
        nc.tensor.transpose(out=po[:], in_=pred_pt[:], identity=ident[:])
        predt_sb = const_pool.tile([NBLK, 128], F32)
        nc.scalar.copy(predt_sb[:], po[:])
        nc.sync.dma_start(pred_t[:], predt_sb[:])
        nc.sync.dma_start(gpre[:], gpre_t[:])

    return nc


_PROGRAM_CACHE = {}


def _get_program(S, NBLK):
    key = (S, NBLK)
    if key not in _PROGRAM_CACHE:
        _PROGRAM_CACHE[key] = _build_program(S, NBLK)
    return _PROGRAM_CACHE[key]


# ---------------------------------------------------------------------------
# Entry point
# ---------------------------------------------------------------------------
def _run(inputs, trace=False, tmpdir=None, trace_cores=None):
    from concourse.bass_utils import run_bass_kernel_spmd

    heads = np.asarray(inputs["heads"]).astype(np.int64)
    tails = np.asarray(inputs["tails"]).astype(np.int64)
    relations = np.asarray(inputs["relations"]).astype(np.int64)
    ent = np.ascontiguousarray(np.asarray(inputs["entity_embedding"], np.float32))
    W = np.asarray(inputs["W"], np.float32)
    V = np.asarray(inputs["V"], np.float32)
    Bp = np.asarray(inputs["Bp"], np.float32)
    U = np.asarray(inputs["U"], np.float32)

    XT = _build_xt(W, V, Bp, U)
    routed, S, NBLK = _route(heads, tails, relations)

    nc = _get_program(S, NBLK)

    in_maps = []
    for c in range(NCORES):
        r = routed[c]
        in_maps.append({
            "xtc": np.ascontiguousarray(XT[r["slot_rels"]].transpose(1, 0, 2)),
            "ent": ent,
            "hsd": r["hsd"],
            "tsd": r["tsd"],
            "scat": r["scat"],
        })

    kwargs = {}
    if trace:
        kwargs.update(trace=True, tmpdir=tmpdir)
        if trace_cores is not None:
            kwargs.update(trace_cores=trace_cores)
    res = run_bass_kernel_spmd(nc, in_maps, core_ids=list(range(NCORES)), **kwargs)

    pred = np.zeros(B, np.float32)
    for c in range(NCORES):
        pt = res.results[c]["pred_t"]  # [NBLK, 128]
        for oi, b, p in routed[c]["placement"]:
            pred[oi] = pt[b, p]
    return pred, routed, res


def kernel(**inputs):
    pred, _, _ = _run(inputs)
    return pred

